# revision 3
# baseline (speedup 1.0000x reference)
"""BEVFusion LSS camera->BEV pooling on 8 Trainium2 NeuronCores.

Strategy (output-voxel sharding):
- Host computes per-point voxel ids + kept mask from the calibration inputs
  (numpy f32, mirroring the reference op-for-op). The big feature tensor is
  never reordered on host: it is sliced per sub-slab (natural point order)
  and padded to 512B rows for dma_gather.
- The BEV grid (129600 voxels) is split into 8*S contiguous voxel ranges
  ("sub-slabs") with ~equal kept-point counts (each <= 32767 points so that
  int16 dma_gather indices reach every row of its sub-slab array).
- Each core processes S sub-slabs: dma_gather fetches its points in
  voxel-sorted order (8192 idx/instruction); each 128-point chunk belongs to
  one 128-voxel window; a one-hot (is_equal vs iota) matmul on the tensor
  engine pools the chunk into PSUM [80ch x 128vox] (exact: weights are
  0.0/1.0); DVE copies PSUM to an SBUF staging ring; blocks stream out to
  DRAM sequentially.
- Host adds the per-chunk blocks into the final [1, 80, 360, 360] grid
  (pure unshard/assembly: block -> its voxel range).
"""
import numpy as np

# ---- problem geometry (hardcoded from the nn.Module config) ----
IMG_H, IMG_W = 256, 704
FH, FW = 32, 88
DBOUND = (1.0, 60.0, 0.5)
XB = (-54.0, 54.0, 0.3)
YB = (-54.0, 54.0, 0.3)
ZB = (-10.0, 10.0, 20.0)
NXX, NXY, NZ = 360, 360, 1
NVOX = NZ * NXX * NXY
C = 80
N_CORES = 8
SUB_CAP = 30000          # max kept points per sub-slab (int16 headroom)
IDX_PER_GATHER = 8192    # HW-validated dma_gather limit
CHUNK = 128
EL = 128                 # padded row length (f32) -> 512B rows

_last_results = None     # test.py introspection


def _compute_coords(lidar2camera, camera_intrinsics):
    """Per-point voxel id + kept mask. Prefer jax on CPU so the float32
    arithmetic matches the reference bit-for-bit (voxel assignment of
    boundary points is sensitive to the einsum's rounding)."""
    try:
        return _compute_coords_jax(lidar2camera, camera_intrinsics)
    except Exception:
        return _compute_coords_np(lidar2camera, camera_intrinsics)


def _compute_coords_jax(lidar2camera, camera_intrinsics):
    import jax
    import jax.numpy as jnp

    with jax.default_device(jax.devices("cpu")[0]):
        l2c = jnp.asarray(np.asarray(lidar2camera, np.float32))
        K = jnp.asarray(np.asarray(camera_intrinsics, np.float32))
        cam2lidar = jnp.linalg.inv(l2c)
        rots = cam2lidar[..., :3, :3]
        trans = cam2lidar[..., :3, 3]
        intrins = K[..., :3, :3]
        ds = jnp.arange(*DBOUND, dtype=jnp.float32)
        D = ds.shape[0]
        xs = jnp.linspace(0.0, IMG_W - 1.0, FW, dtype=jnp.float32)
        ys = jnp.linspace(0.0, IMG_H - 1.0, FH, dtype=jnp.float32)
        ds_b = jnp.broadcast_to(ds[:, None, None], (D, FH, FW))
        xs_b = jnp.broadcast_to(xs[None, None, :], (D, FH, FW))
        ys_b = jnp.broadcast_to(ys[None, :, None], (D, FH, FW))
        frustum = jnp.stack((xs_b, ys_b, ds_b), axis=-1)
        pts = jnp.concatenate(
            [frustum[..., :2] * frustum[..., 2:3], frustum[..., 2:3]], axis=-1
        )
        combine = rots @ jnp.linalg.inv(intrins)
        geom = jnp.einsum("bnij,dhwj->bndhwi", combine, pts) + trans[
            :, :, None, None, None, :
        ]
        DX = jnp.array([XB[2], YB[2], ZB[2]], jnp.float32)
        BX = jnp.array(
            [XB[0] + XB[2] / 2.0, YB[0] + YB[2] / 2.0, ZB[0] + ZB[2] / 2.0],
            jnp.float32,
        )
        B, N = l2c.shape[0], l2c.shape[1]
        Nprime = B * N * D * FH * FW
        coords = ((geom.reshape(Nprime, 3) - (BX - DX / 2.0)) / DX).astype(jnp.int32)
        kept = (
            (coords[:, 0] >= 0) & (coords[:, 0] < NXX)
            & (coords[:, 1] >= 0) & (coords[:, 1] < NXY)
            & (coords[:, 2] >= 0) & (coords[:, 2] < NZ)
        )
        flat = (coords[:, 2] * NXX + coords[:, 0]) * NXY + coords[:, 1]
        return np.asarray(flat).astype(np.int64), np.asarray(kept)


def _compute_coords_np(lidar2camera, camera_intrinsics):
    l2c = np.asarray(lidar2camera, dtype=np.float32)
    K = np.asarray(camera_intrinsics, dtype=np.float32)
    cam2lidar = np.linalg.inv(l2c)
    rots = cam2lidar[..., :3, :3]
    trans = cam2lidar[..., :3, 3]
    intrins = K[..., :3, :3]

    ds = np.arange(*DBOUND, dtype=np.float32)
    D = ds.shape[0]
    xs = np.linspace(0.0, IMG_W - 1.0, FW, dtype=np.float32)
    ys = np.linspace(0.0, IMG_H - 1.0, FH, dtype=np.float32)
    ds_b = np.broadcast_to(ds[:, None, None], (D, FH, FW))
    xs_b = np.broadcast_to(xs[None, None, :], (D, FH, FW))
    ys_b = np.broadcast_to(ys[None, :, None], (D, FH, FW))
    frustum = np.stack((xs_b, ys_b, ds_b), axis=-1)
    pts = np.concatenate(
        [frustum[..., :2] * frustum[..., 2:3], frustum[..., 2:3]], axis=-1
    ).astype(np.float32)
    combine = (rots @ np.linalg.inv(intrins)).astype(np.float32)
    geom = np.einsum("bnij,dhwj->bndhwi", combine, pts, dtype=np.float32) + trans[
        :, :, None, None, None, :
    ]
    DX = np.array([XB[2], YB[2], ZB[2]], np.float32)
    BX = np.array(
        [XB[0] + XB[2] / 2.0, YB[0] + YB[2] / 2.0, ZB[0] + ZB[2] / 2.0], np.float32
    )
    B, N = l2c.shape[0], l2c.shape[1]
    Nprime = B * N * D * FH * FW
    coords = ((geom.reshape(Nprime, 3) - (BX - DX / 2.0)) / DX).astype(np.int32)
    kept = (
        (coords[:, 0] >= 0) & (coords[:, 0] < NXX)
        & (coords[:, 1] >= 0) & (coords[:, 1] < NXY)
        & (coords[:, 2] >= 0) & (coords[:, 2] < NZ)
    )
    flat = (coords[:, 2].astype(np.int64) * NXX + coords[:, 0]) * NXY + coords[:, 1]
    return flat, kept


def _plan(vox, kept):
    """Partition voxels into 8*S ranges with ~equal kept counts; build the
    voxel-sorted, chunk-padded gather streams per sub-slab."""
    T = int(kept.sum())
    rows_all = np.nonzero(kept)[0]
    v_kept = vox[rows_all]
    counts = np.bincount(v_kept, minlength=NVOX)
    csum = np.cumsum(counts)
    s_per_core = max(1, int(np.ceil(T / SUB_CAP / N_CORES)))
    nsub = N_CORES * s_per_core
    bounds = [0]
    for i in range(1, nsub):
        bounds.append(int(np.searchsorted(csum, T * i // nsub)))
    bounds.append(NVOX)

    subs = []
    for s in range(nsub):
        lo, hi = bounds[s], bounds[s + 1]
        sel = (v_kept >= lo) & (v_kept < hi)
        rows_s = rows_all[sel]          # natural order positions into x2d
        v_s = v_kept[sel]
        n_s = len(rows_s)
        assert n_s <= 32767, f"sub-slab {s} has {n_s} points"
        order = np.argsort(v_s, kind="stable")
        v_sorted = v_s[order]
        gw = (v_sorted - lo) >> 7
        n_gw = ((hi - lo) + 127) >> 7 if hi > lo else 0
        sizes = np.bincount(gw, minlength=max(n_gw, 1)) if n_s else np.zeros(max(n_gw, 1), np.int64)
        cpg = (sizes + CHUNK - 1) // CHUNK            # chunks per gw
        cbase = np.concatenate([[0], np.cumsum(cpg)])
        nchunks = int(cbase[-1])
        # rank of each sorted point within its gw group
        gstart = np.concatenate([[0], np.cumsum(sizes)])
        ranks = np.arange(n_s, dtype=np.int64) - gstart[gw]
        pos = cbase[gw] * CHUNK + ranks
        stream_idx = np.zeros(nchunks * CHUNK, np.int16)
        stream_slot = np.full(nchunks * CHUNK, 255, np.float32)
        stream_idx[pos] = order.astype(np.int16)
        stream_slot[pos] = ((v_sorted - lo) & 127).astype(np.float32)
        gw_of_chunk = np.repeat(np.arange(len(cpg), dtype=np.int64), cpg)
        subs.append(dict(lo=lo, hi=hi, rows=rows_s, nchunks=nchunks,
                         stream_idx=stream_idx, stream_slot=stream_slot,
                         gw_of_chunk=gw_of_chunk))
    return subs, s_per_core, bounds


def _build_and_run(x2d, subs, s_per_core):
    import concourse.bass as bass
    import concourse.bacc as bacc
    import concourse.mybir as mybir
    import concourse.tile as tile
    from concourse.bass_utils import run_bass_kernel_spmd

    S = s_per_core
    nsub = len(subs)
    nmax = max(len(sb["rows"]) for sb in subs)
    NSUB_MAX = min(32767, ((nmax + 127) // 128) * 128)
    assert nmax <= NSUB_MAX
    G0 = max(sb["nchunks"] for sb in subs)
    G0 = ((G0 + 63) // 64) * 64                       # align to gather instr
    NGATH = G0 // 64                                  # gathers per sub-slab
    NBLK = S * G0                                     # out blocks per core

    # ---- per-core input tensors ----
    in_maps = []
    gw_maps = []  # per core: list of (real, gw) per global block
    for k in range(N_CORES):
        xs = np.zeros((S, NSUB_MAX, EL), np.float32)
        idxs = np.zeros((S, NGATH, 128, IDX_PER_GATHER // 16), np.int16)
        slots = np.full((128, S * G0), 255.0, np.float32)
        gmap = []
        for v in range(S):
            sb = subs[k * S + v]
            n_s = len(sb["rows"])
            xs[v, :n_s, :C] = x2d[sb["rows"]]
            si = np.zeros(G0 * CHUNK, np.int16)
            sl = np.full(G0 * CHUNK, 255.0, np.float32)
            ln = sb["nchunks"] * CHUNK
            si[:ln] = sb["stream_idx"]
            sl[:ln] = sb["stream_slot"]
            # idx wrap: entry i -> [i%16, i//16], replicated to 8 groups
            w = si.reshape(NGATH, IDX_PER_GATHER // 16, 16).transpose(0, 2, 1)
            idxs[v] = np.tile(w, (1, 8, 1))
            # slot table: point (p, j) = stream position j*128+p
            slots[:, v * G0:(v + 1) * G0] = sl.reshape(G0, CHUNK).T
            for j in range(G0):
                gmap.append((j < sb["nchunks"], sb["lo"] + int(sb["gw_of_chunk"][j]) * 128 if j < sb["nchunks"] else 0))
        iota = np.broadcast_to(np.arange(128, dtype=np.float32), (128, 128)).copy()
        in_maps.append({"xs": xs, "idxs": idxs, "slots": slots, "iota": iota})
        gw_maps.append(gmap)

    # ---- build the SPMD program ----
    nc = bacc.Bacc("TRN2", target_bir_lowering=False, debug=False,
                   num_devices=N_CORES)
    xs_d = nc.declare_dram_parameter("xs", [S, NSUB_MAX, EL], mybir.dt.float32, isOutput=False)
    idxs_d = nc.declare_dram_parameter("idxs", [S, NGATH, 128, IDX_PER_GATHER // 16], mybir.dt.int16, isOutput=False)
    slots_d = nc.declare_dram_parameter("slots", [128, S * G0], mybir.dt.float32, isOutput=False)
    iota_d = nc.declare_dram_parameter("iota", [128, 128], mybir.dt.float32, isOutput=False)
    out_d = nc.declare_dram_parameter("out", [80, NBLK * 128], mybir.dt.float32, isOutput=True)

    SB = 16  # staging ring blocks; DMA out every SB//2
    with tile.TileContext(nc) as tc:
        with (
            tc.tile_pool(name="io", bufs=1) as io_pool,
            tc.tile_pool(name="gather", bufs=2) as g_pool,
            tc.tile_pool(name="oh", bufs=4) as oh_pool,
            tc.tile_pool(name="stage", bufs=2) as st_pool,
            tc.tile_pool(name="psum", bufs=8, space="PSUM") as ps_pool,
        ):
            slot_t = io_pool.tile([128, S * G0], mybir.dt.float32, tag="slots")
            nc.sync.dma_start(out=slot_t[:], in_=slots_d[:])
            iota_t = io_pool.tile([128, 128], mybir.dt.float32, tag="iota")
            nc.sync.dma_start(out=iota_t[:], in_=iota_d[:])
            idx_t = io_pool.tile([128, S * NGATH * (IDX_PER_GATHER // 16)], mybir.dt.int16, tag="idx")
            for v in range(S):
                for g in range(NGATH):
                    o = (v * NGATH + g) * (IDX_PER_GATHER // 16)
                    nc.sync.dma_start(
                        out=idx_t[:, o:o + IDX_PER_GATHER // 16],
                        in_=idxs_d[v, g],
                    )

            blk = 0
            stage_t = None
            for v in range(S):
                for g in range(NGATH):
                    gt = g_pool.tile([128, 64 * EL], mybir.dt.float32, tag="gt")
                    o = (v * NGATH + g) * (IDX_PER_GATHER // 16)
                    nc.gpsimd.dma_gather(
                        out_ap=gt[:].rearrange("p (j e) -> p j e", e=EL),
                        in_ap=xs_d[v],
                        idxs_ap=idx_t[:, o:o + IDX_PER_GATHER // 16],
                        num_idxs=IDX_PER_GATHER,
                        num_idxs_reg=IDX_PER_GATHER,
                        elem_size=EL,
                        single_packet=False,
                    )
                    for j64 in range(64):
                        J = v * G0 + g * 64 + j64
                        oh = oh_pool.tile([128, 128], mybir.dt.float32, tag="oh")
                        nc.vector.tensor_tensor(
                            out=oh[:],
                            in0=slot_t[:, J:J + 1].to_broadcast([128, 128]),
                            in1=iota_t[:],
                            op=mybir.AluOpType.is_equal,
                        )
                        ps = ps_pool.tile([80, 128], mybir.dt.float32, tag="ps")
                        nc.tensor.matmul(
                            out=ps[:],
                            lhsT=gt[:].rearrange("p (j e) -> p j e", e=EL)[:, j64, 0:C],
                            rhs=oh[:],
                            start=True,
                            stop=True,
                        )
                        if blk % SB == 0:
                            stage_t = st_pool.tile([80, SB * 128], mybir.dt.float32, tag="st")
                        r = blk % SB
                        nc.vector.tensor_copy(
                            out=stage_t[:, r * 128:(r + 1) * 128], in_=ps[:]
                        )
                        blk += 1
                        if blk % SB == 0:
                            nc.sync.dma_start(
                                out=out_d[:, (blk - SB) * 128:blk * 128],
                                in_=stage_t[:],
                            )
            assert blk % SB == 0, f"NBLK {NBLK} not multiple of {SB}"

    nc.compile()
    res = run_bass_kernel_spmd(nc, in_maps, core_ids=list(range(N_CORES)))
    global _last_results
    _last_results = res
    return res, gw_maps, NBLK


def kernel(x, lidar2camera, camera_intrinsics):
    x = np.asarray(x)
    B, N, D, H, W, C_ = x.shape
    assert (B, N, H, W, C_) == (1, 6, FH, FW, C), x.shape
    vox, kept = _compute_coords(lidar2camera, camera_intrinsics)
    subs, s_per_core, bounds = _plan(vox, kept)
    x2d = np.ascontiguousarray(x.reshape(-1, C))
    res, gw_maps, NBLK = _build_and_run(x2d, subs, s_per_core)

    grid = np.zeros((C, NVOX), np.float32)
    for k in range(N_CORES):
        out_k = res.results[k]["out"]
        for J, (real, base) in enumerate(gw_maps[k]):
            if not real:
                continue
            e = min(base + 128, NVOX)
            grid[:, base:e] += out_k[:, J * 128:J * 128 + (e - base)]
    return grid.reshape(1, C * NZ, NXX, NXY)


# revision 4
# speedup vs baseline: 2.3842x; 2.3842x over previous
"""BEVFusion LSS camera->BEV pooling on 8 Trainium2 NeuronCores.

Strategy (output-voxel sharding):
- Host computes per-point voxel ids + kept mask from the calibration inputs
  (jax on CPU, mirroring the reference op-for-op so voxel assignment of
  boundary points matches bit-for-bit; numpy fallback). The big feature
  tensor is never reordered on host: it is sliced per sub-slab (natural
  point order) and padded to 512B rows for dma_gather.
- Kept points are conceptually sorted by voxel and grouped into 128-point
  chunks, each chunk belonging to one 128-voxel grid window (gw); the global
  chunk stream is cut into 8*S equal ranges ("sub-slabs", <=30976 points
  each so int16 dma_gather indices reach every row of the sub-slab array).
- Each core processes S sub-slabs: dma_gather (4 SWDGE queues round-robin)
  fetches its points in voxel-sorted order; a one-hot (is_equal vs iota)
  matmul on the tensor engine pools each chunk into PSUM [80ch x 512]
  (4 chunks per PSUM bank; weights are exactly 0.0/1.0); DVE copies PSUM to
  an SBUF staging ring; blocks stream out to DRAM sequentially.
- Host adds the per-chunk blocks into the final [1, 80, 360, 360] grid
  (pure unshard/assembly: each block -> its gw's voxel range).
"""
import numpy as np

# ---- problem geometry (hardcoded from the nn.Module config) ----
IMG_H, IMG_W = 256, 704
FH, FW = 32, 88
DBOUND = (1.0, 60.0, 0.5)
XB = (-54.0, 54.0, 0.3)
YB = (-54.0, 54.0, 0.3)
ZB = (-10.0, 10.0, 20.0)
NXX, NXY, NZ = 360, 360, 1
NVOX = NZ * NXX * NXY
NGW = (NVOX + 127) // 128
C = 80
N_CORES = 8
CHUNK_CAP = 242          # chunks per sub-slab target (242*128 = 30976 <= 32767)
IDX_PER_GATHER = 8192    # HW-validated dma_gather limit
CHUNK = 128
EL = 128                 # padded row length (f32) -> 512B rows

_last_results = None     # test.py introspection


def _compute_coords(lidar2camera, camera_intrinsics):
    try:
        return _compute_coords_jax(lidar2camera, camera_intrinsics)
    except Exception:
        return _compute_coords_np(lidar2camera, camera_intrinsics)


def _compute_coords_jax(lidar2camera, camera_intrinsics):
    import jax
    import jax.numpy as jnp

    with jax.default_device(jax.devices("cpu")[0]):
        l2c = jnp.asarray(np.asarray(lidar2camera, np.float32))
        K = jnp.asarray(np.asarray(camera_intrinsics, np.float32))
        cam2lidar = jnp.linalg.inv(l2c)
        rots = cam2lidar[..., :3, :3]
        trans = cam2lidar[..., :3, 3]
        intrins = K[..., :3, :3]
        ds = jnp.arange(*DBOUND, dtype=jnp.float32)
        D = ds.shape[0]
        xs = jnp.linspace(0.0, IMG_W - 1.0, FW, dtype=jnp.float32)
        ys = jnp.linspace(0.0, IMG_H - 1.0, FH, dtype=jnp.float32)
        ds_b = jnp.broadcast_to(ds[:, None, None], (D, FH, FW))
        xs_b = jnp.broadcast_to(xs[None, None, :], (D, FH, FW))
        ys_b = jnp.broadcast_to(ys[None, :, None], (D, FH, FW))
        frustum = jnp.stack((xs_b, ys_b, ds_b), axis=-1)
        pts = jnp.concatenate(
            [frustum[..., :2] * frustum[..., 2:3], frustum[..., 2:3]], axis=-1
        )
        combine = rots @ jnp.linalg.inv(intrins)
        geom = jnp.einsum("bnij,dhwj->bndhwi", combine, pts) + trans[
            :, :, None, None, None, :
        ]
        DX = jnp.array([XB[2], YB[2], ZB[2]], jnp.float32)
        BX = jnp.array(
            [XB[0] + XB[2] / 2.0, YB[0] + YB[2] / 2.0, ZB[0] + ZB[2] / 2.0],
            jnp.float32,
        )
        B, N = l2c.shape[0], l2c.shape[1]
        Nprime = B * N * D * FH * FW
        coords = ((geom.reshape(Nprime, 3) - (BX - DX / 2.0)) / DX).astype(jnp.int32)
        kept = (
            (coords[:, 0] >= 0) & (coords[:, 0] < NXX)
            & (coords[:, 1] >= 0) & (coords[:, 1] < NXY)
            & (coords[:, 2] >= 0) & (coords[:, 2] < NZ)
        )
        flat = (coords[:, 2] * NXX + coords[:, 0]) * NXY + coords[:, 1]
        return np.asarray(flat).astype(np.int64), np.asarray(kept)


def _compute_coords_np(lidar2camera, camera_intrinsics):
    l2c = np.asarray(lidar2camera, dtype=np.float32)
    K = np.asarray(camera_intrinsics, dtype=np.float32)
    cam2lidar = np.linalg.inv(l2c)
    rots = cam2lidar[..., :3, :3]
    trans = cam2lidar[..., :3, 3]
    intrins = K[..., :3, :3]
    ds = np.arange(*DBOUND, dtype=np.float32)
    D = ds.shape[0]
    xs = np.linspace(0.0, IMG_W - 1.0, FW, dtype=np.float32)
    ys = np.linspace(0.0, IMG_H - 1.0, FH, dtype=np.float32)
    ds_b = np.broadcast_to(ds[:, None, None], (D, FH, FW))
    xs_b = np.broadcast_to(xs[None, None, :], (D, FH, FW))
    ys_b = np.broadcast_to(ys[None, :, None], (D, FH, FW))
    frustum = np.stack((xs_b, ys_b, ds_b), axis=-1)
    pts = np.concatenate(
        [frustum[..., :2] * frustum[..., 2:3], frustum[..., 2:3]], axis=-1
    ).astype(np.float32)
    combine = (rots @ np.linalg.inv(intrins)).astype(np.float32)
    geom = np.einsum("bnij,dhwj->bndhwi", combine, pts, dtype=np.float32) + trans[
        :, :, None, None, None, :
    ]
    DX = np.array([XB[2], YB[2], ZB[2]], np.float32)
    BX = np.array(
        [XB[0] + XB[2] / 2.0, YB[0] + YB[2] / 2.0, ZB[0] + ZB[2] / 2.0], np.float32
    )
    B, N = l2c.shape[0], l2c.shape[1]
    Nprime = B * N * D * FH * FW
    coords = ((geom.reshape(Nprime, 3) - (BX - DX / 2.0)) / DX).astype(np.int32)
    kept = (
        (coords[:, 0] >= 0) & (coords[:, 0] < NXX)
        & (coords[:, 1] >= 0) & (coords[:, 1] < NXY)
        & (coords[:, 2] >= 0) & (coords[:, 2] < NZ)
    )
    flat = (coords[:, 2].astype(np.int64) * NXX + coords[:, 0]) * NXY + coords[:, 1]
    return flat, kept


def _plan(vox, kept):
    """Global voxel-sorted chunk stream, cut into 8*S equal sub-slabs."""
    rows_all = np.nonzero(kept)[0]
    v_kept = vox[rows_all]
    order = np.argsort(v_kept, kind="stable")
    v_sorted = v_kept[order]
    rows_sorted = rows_all[order]
    gw = v_sorted >> 7
    slot = (v_sorted & 127).astype(np.float32)
    sizes = np.bincount(gw, minlength=NGW)
    cpg = (sizes + CHUNK - 1) // CHUNK
    cbase = np.concatenate([[0], np.cumsum(cpg)])
    total_chunks = int(cbase[-1])
    gstart = np.concatenate([[0], np.cumsum(sizes)])
    ranks = np.arange(len(v_sorted), dtype=np.int64) - gstart[gw]
    pos = cbase[gw] * CHUNK + ranks
    stream_row = np.full(total_chunks * CHUNK, -1, np.int64)
    stream_slot = np.full(total_chunks * CHUNK, 255.0, np.float32)
    stream_row[pos] = rows_sorted
    stream_slot[pos] = slot
    gw_of_chunk = np.repeat(np.arange(NGW, dtype=np.int64), cpg)

    s_per_core = max(1, int(np.ceil(total_chunks / CHUNK_CAP / N_CORES)))
    nsub = N_CORES * s_per_core
    Q = (total_chunks + nsub - 1) // nsub
    G0 = ((Q + 63) // 64) * 64

    subs = []
    for s in range(nsub):
        clo, chi = s * Q, min((s + 1) * Q, total_chunks)
        nch = max(0, chi - clo)
        sr = stream_row[clo * CHUNK:chi * CHUNK]
        ss = stream_slot[clo * CHUNK:chi * CHUNK]
        valid = sr >= 0
        rows_used = np.unique(sr[valid])  # ascending = natural order
        loc = np.zeros(len(sr), np.int16)
        loc[valid] = np.searchsorted(rows_used, sr[valid]).astype(np.int16)
        subs.append(dict(rows=rows_used, nchunks=nch, idx=loc, slot=ss,
                         gw=gw_of_chunk[clo:chi]))
    return subs, s_per_core, G0


def _build_and_run(x2d, subs, s_per_core, G0):
    import concourse.bass as bass
    import concourse.bacc as bacc
    import concourse.mybir as mybir
    import concourse.tile as tile
    from concourse.bass_utils import run_bass_kernel_spmd

    S = s_per_core
    nmax = max(len(sb["rows"]) for sb in subs)
    NSUB_MAX = min(32767, ((nmax + 127) // 128) * 128)
    assert nmax <= NSUB_MAX
    NGATH = G0 // 64
    NBLK = S * G0

    in_maps = []
    gw_maps = []
    for k in range(N_CORES):
        xs = np.zeros((S, NSUB_MAX, EL), np.float32)
        idxs = np.zeros((S, NGATH, 128, IDX_PER_GATHER // 16), np.int16)
        slots = np.full((128, S * G0), 255.0, np.float32)
        gmap = []
        for v in range(S):
            sb = subs[k * S + v]
            n_s = len(sb["rows"])
            xs[v, :n_s, :C] = x2d[sb["rows"]]
            si = np.zeros(G0 * CHUNK, np.int16)
            sl = np.full(G0 * CHUNK, 255.0, np.float32)
            ln = sb["nchunks"] * CHUNK
            si[:ln] = sb["idx"]
            sl[:ln] = sb["slot"]
            w = si.reshape(NGATH, IDX_PER_GATHER // 16, 16).transpose(0, 2, 1)
            idxs[v] = np.tile(w, (1, 8, 1))
            slots[:, v * G0:(v + 1) * G0] = sl.reshape(G0, CHUNK).T
            for j in range(G0):
                if j < sb["nchunks"]:
                    gmap.append(int(sb["gw"][j]) * 128)
                else:
                    gmap.append(-1)
        iota4 = np.tile(np.arange(128, dtype=np.float32), (128, 4)).copy()
        in_maps.append({"xs": xs, "idxs": idxs, "slots": slots, "iota": iota4})
        gw_maps.append(gmap)

    nc = bacc.Bacc("TRN2", target_bir_lowering=False, debug=False,
                   num_devices=N_CORES, num_swdge_queues=4)
    xs_d = nc.declare_dram_parameter("xs", [S, NSUB_MAX, EL], mybir.dt.float32, isOutput=False)
    idxs_d = nc.declare_dram_parameter("idxs", [S, NGATH, 128, IDX_PER_GATHER // 16], mybir.dt.int16, isOutput=False)
    slots_d = nc.declare_dram_parameter("slots", [128, S * G0], mybir.dt.float32, isOutput=False)
    iota_d = nc.declare_dram_parameter("iota", [128, 4 * 128], mybir.dt.float32, isOutput=False)
    out_d = nc.declare_dram_parameter("out", [80, NBLK * 128], mybir.dt.float32, isOutput=True)

    SB = 16  # staging ring blocks (4 psum batches)
    with tile.TileContext(nc) as tc:
        with (
            tc.tile_pool(name="io", bufs=1) as io_pool,
            tc.tile_pool(name="gather", bufs=3) as g_pool,
            tc.tile_pool(name="oh", bufs=4) as oh_pool,
            tc.tile_pool(name="stage", bufs=3) as st_pool,
            tc.tile_pool(name="psum", bufs=6, space="PSUM") as ps_pool,
        ):
            slot_t = io_pool.tile([128, S * G0], mybir.dt.float32, tag="slots")
            nc.sync.dma_start(out=slot_t[:], in_=slots_d[:])
            iota_t = io_pool.tile([128, 4 * 128], mybir.dt.float32, tag="iota")
            nc.sync.dma_start(out=iota_t[:], in_=iota_d[:])
            idx_t = io_pool.tile([128, S * NGATH * (IDX_PER_GATHER // 16)], mybir.dt.int16, tag="idx")
            for v in range(S):
                for g in range(NGATH):
                    o = (v * NGATH + g) * (IDX_PER_GATHER // 16)
                    nc.sync.dma_start(
                        out=idx_t[:, o:o + IDX_PER_GATHER // 16],
                        in_=idxs_d[v, g],
                    )

            blk = 0
            stage_t = None
            for v in range(S):
                for g in range(NGATH):
                    gt = g_pool.tile([128, 64 * EL], mybir.dt.float32, tag="gt")
                    o = (v * NGATH + g) * (IDX_PER_GATHER // 16)
                    nc.gpsimd.dma_gather(
                        out_ap=gt[:].rearrange("p (j e) -> p j e", e=EL),
                        in_ap=xs_d[v],
                        idxs_ap=idx_t[:, o:o + IDX_PER_GATHER // 16],
                        num_idxs=IDX_PER_GATHER,
                        num_idxs_reg=IDX_PER_GATHER,
                        elem_size=EL,
                        single_packet=False,
                        queue_num=(v * NGATH + g) % 4,
                    )
                    for q4 in range(16):  # 16 batches of 4 chunks
                        J0 = v * G0 + g * 64 + q4 * 4
                        oh = oh_pool.tile([128, 4 * 128], mybir.dt.float32, tag="oh")
                        nc.vector.tensor_tensor(
                            out=oh[:].rearrange("p (f s) -> p f s", s=128),
                            in0=slot_t[:, J0:J0 + 4].to_broadcast([128, 4, 128]),
                            in1=iota_t[:].rearrange("p (f s) -> p f s", s=128),
                            op=mybir.AluOpType.is_equal,
                        )
                        ps = ps_pool.tile([80, 512], mybir.dt.float32, tag="ps")
                        for jj in range(4):
                            j64 = q4 * 4 + jj
                            nc.tensor.matmul(
                                out=ps[:, jj * 128:(jj + 1) * 128],
                                lhsT=gt[:].rearrange("p (j e) -> p j e", e=EL)[:, j64, 0:C],
                                rhs=oh[:, jj * 128:(jj + 1) * 128],
                                start=True,
                                stop=True,
                            )
                        if blk % SB == 0:
                            stage_t = st_pool.tile([80, SB * 128], mybir.dt.float32, tag="st")
                        r = blk % SB
                        nc.vector.tensor_copy(
                            out=stage_t[:, r * 128:(r + 4) * 128], in_=ps[:]
                        )
                        blk += 4
                        if blk % SB == 0:
                            nc.sync.dma_start(
                                out=out_d[:, (blk - SB) * 128:blk * 128],
                                in_=stage_t[:],
                            )
            assert blk % SB == 0, f"NBLK {NBLK} not multiple of {SB}"

    nc.compile()
    res = run_bass_kernel_spmd(nc, in_maps, core_ids=list(range(N_CORES)))
    global _last_results
    _last_results = res
    return res, gw_maps


def kernel(x, lidar2camera, camera_intrinsics):
    x = np.asarray(x)
    B, N, D, H, W, C_ = x.shape
    assert (B, N, H, W, C_) == (1, 6, FH, FW, C), x.shape
    vox, kept = _compute_coords(lidar2camera, camera_intrinsics)
    subs, s_per_core, G0 = _plan(vox, kept)
    x2d = np.ascontiguousarray(x.reshape(-1, C))
    res, gw_maps = _build_and_run(x2d, subs, s_per_core, G0)

    grid = np.zeros((C, NVOX), np.float32)
    for k in range(N_CORES):
        out_k = res.results[k]["out"]
        for J, base in enumerate(gw_maps[k]):
            if base < 0:
                continue
            e = min(base + 128, NVOX)
            grid[:, base:e] += out_k[:, J * 128:J * 128 + (e - base)]
    return grid.reshape(1, C * NZ, NXX, NXY)
